# revision 48
# baseline (speedup 1.0000x reference)
"""Trainium2 Bass kernel for a pre-LN transformer block (B=2, S=2048, D=1024,
H=16, d_ff=4096), 8-way (batch, head-group) tensor-parallel:

- core c handles batch c//4 and heads 4*(c%4)..4*(c%4)+3: LN1+qkv run over the
  core's 2048 batch tokens only, attention over 4 heads
- softmax exp is split across engines: even key-tiles use the Activation
  engine's exact Exp, odd key-tiles use a Schraudolph-style int16 exponent
  construction on the DVE (bitcast to bf16)
- attention-proj partials are ReduceScattered per query-chunk (4 collectives),
  each fired as soon as that chunk's proj partials are done, so 3 of 4 overlap
  the remaining attention compute; each core owns four interleaved 128-token
  slabs (slab qc = tokens qc*512 + rank*128 ..+128) so the residual+LN2+MLP
  pipeline starts at attention end, with the MLP split into two 256-token
  passes (the second gated only on the last collective)
- token-sharded MLP with the full d_ff on each core (no second collective)

Activations live feature-major [feature, token].  LayerNorm is folded into the
matmuls via an augmented contraction row (-mu) and column (row-sums of the
g-scaled weights); the 1/sigma factor is applied on PSUM eviction.  Softmax is
computed unnormalized with a ones-column appended to V producing row sums, and
1/sum is applied on the attention-output eviction.
"""

import sys

for _p in ("/opt/trn_rl_repo",):
    if _p not in sys.path:
        sys.path.insert(0, _p)

import numpy as np
import ml_dtypes

B, S, D = 2, 2048, 1024
H, HD = 16, 64
FF = 4 * D
T = B * S  # 4096 tokens
NCORES = 8
TC = T // NCORES  # 512 tokens per core (MLP/out shard)
TB = S  # 2048 tokens per batch (per-core attention range)
P = 128
KT = D // P  # 8 k-tiles over D
KA = 9  # augmented k-tiles
DAUG = D + P  # 1152
EPS = 1e-5
NKT = TB // P  # 16 key tiles per batch
NQC = TB // 512  # 4 q-chunks of 512
SLAB = TC // NQC  # 128 tokens per owned slab
BF16 = ml_dtypes.bfloat16

# Schraudolph exp: bf16 bits ~= round(x*log2(e)*128 + (127*128 - 7.63))
LOG2E = float(np.log2(np.e))
EXP_A = 128.0 * LOG2E / np.sqrt(HD)  # logit scale 1/sqrt(HD) folded in
EXP_B = 127.0 * 128.0 - 7.63
# key tiles using exact Exp on the Activation engine (rest: Schraudolph on DVE)
SC_KT = frozenset({0, 2, 4, 6, 8, 10, 12, 14})

_CACHE = {}


def _build_program(has_c1, has_bproj, has_c2, has_b1, has_b2):
    import concourse.mybir as mybir
    import concourse.tile as tile
    from concourse import bacc
    from concourse.masks import make_identity
    from contextlib import ExitStack

    f32 = mybir.dt.float32
    bf16 = mybir.dt.bfloat16
    f8 = mybir.dt.float8e4
    i16 = mybir.dt.int16
    AF = mybir.ActivationFunctionType
    ALU = mybir.AluOpType

    nc = bacc.Bacc(None, target_bir_lowering=False)

    # ---- I/O ----
    x_aug_d = nc.declare_dram_parameter("x_aug", [DAUG, TB], bf16, isOutput=False)
    x_c_d = nc.declare_dram_parameter("x_c", [D, TC], bf16, isOutput=False)
    wqkv_d = nc.declare_dram_parameter("wqkv_aug", [DAUG, 6 * P], bf16, isOutput=False)
    wproj_d = nc.declare_dram_parameter("wproj_c", [2 * P, D], bf16, isOutput=False)
    w1_d = nc.declare_dram_parameter("w1_aug", [D, FF], bf16, isOutput=False)
    w2t_d = nc.declare_dram_parameter("w2t", [FF, D], bf16, isOutput=False)
    aux_d = nc.declare_dram_parameter("aux", [P, 64], f32, isOutput=False)
    # aux columns: 0:8 -> b_proj as [128,8], 8:40 -> b1 as [128,32],
    # 40:48 -> b2 as [128,8], 48:54 -> C1 (qkv bias-fold) as [128,6]
    out_d = nc.declare_dram_parameter("out_c", [D, TC], f32, isOutput=True)

    groups = [[0, 1, 2, 3], [4, 5, 6, 7]]

    with tile.TileContext(nc) as tc, ExitStack() as ctx:
        const = ctx.enter_context(tc.tile_pool(name="const", bufs=1))
        dram = ctx.enter_context(tc.tile_pool(name="dram", bufs=1, space="DRAM"))

        ident = const.tile([P, P], bf16)
        make_identity(nc, ident)
        ones128 = const.tile([P, P], bf16)
        nc.any.memset(ones128, 1.0)
        eps_col = const.tile([P, 1], f32)
        nc.any.memset(eps_col, EPS)

        wqkv_sb = const.tile([P, KA, 6 * P], bf16)
        nc.sync.dma_start(wqkv_sb, wqkv_d.rearrange("(k p) e -> p k e", p=P))
        wproj_sb = const.tile([P, 2, D], bf16)
        nc.sync.dma_start(wproj_sb, wproj_d.rearrange("(k p) d -> p k d", p=P))
        aux_sb = const.tile([P, 64], f32)
        nc.sync.dma_start(aux_sb, aux_d[:])

        # long-lived activation tensors
        x1grp = ctx.enter_context(tc.tile_pool(name="x1grp", bufs=1))
        x1aug = x1grp.tile([P, KT, TC], bf16)
        work = ctx.enter_context(tc.tile_pool(name="work", bufs=1))

        psA = ctx.enter_context(tc.tile_pool(name="psA", bufs=2, space="PSUM"))

        # residual input, prefetched during attention
        resid = ctx.enter_context(tc.tile_pool(name="resid", bufs=1))
        xc = resid.tile([P, KT, TC], bf16, tag="xc")
        xb = resid.tile([P, KT, TC], bf16, tag="xb")

        # w1 weights, prefetched during attention
        w1_pool = ctx.enter_context(tc.tile_pool(name="w1pool", bufs=1))

        # proj partials per query chunk, wide-row layout for the collective:
        # row r*128 + p, col m*128 + t  <->  feature m*128+p, rank-r slab
        # token t (2KB rows so the ReduceScatter moves efficient lines)
        # fp8 partials (w_proj is pre-scaled x64 on the host so values sit in
        # e4m3's good range); the gpsimd cast-DMA converts back to bf16 and
        # the slab pipeline undoes the x64
        partial_d = [
            dram.tile([4 * P, KT * SLAB], f8, tag=f"pp{qc}", name=f"pp{qc}")
            for qc in range(NQC)
        ]
        x1p_d = [
            dram.tile([P, KT * SLAB], f8, tag=f"xp{qc}", name=f"xp{qc}")
            for qc in range(NQC)
        ]

        x_aug_r = x_aug_d.rearrange("(k p) t -> p k t", p=P)
        w1_noaug_r = w1_d.rearrange("(k p) f -> p k f", p=P)

        w1q = []
        with tc.tile_pool(name="qkvTp", bufs=1) as qkvT_pool, \
             tc.tile_pool(name="attnTp", bufs=1) as attnT_pool, \
             tc.tile_pool(name="attg", bufs=1) as attg, \
             tc.tile_pool(name="etp", bufs=9) as etp, \
             tc.tile_pool(name="poutp", bufs=3) as poutp, \
             tc.tile_pool(name="lgp", bufs=3, space="PSUM") as lgp, \
             tc.tile_pool(name="avqp", bufs=3, space="PSUM") as avqp:
            qkvT = [qkvT_pool.tile([P, 2, TB], bf16, name=f"qkvT{pt}") for pt in (0, 1)]
            attnT = [attnT_pool.tile([P, TB], bf16, name=f"attnT{pt}") for pt in (0, 1)]
            # vext: per key tile: [h0 | 1 | h1 | 1 | h2 | 1 | h3 | 1]
            vext = attg.tile([P, NKT, 4 * 65], bf16)

            # ============ phase A: LN1 stats + qkv + vext, per token chunk ===
            with tc.tile_pool(name="xaug", bufs=2) as xaug_pool, \
                 tc.tile_pool(name="workA", bufs=2) as workA, \
                 nc.named_scope("ln1_qkv"):
                for hp in range(4):
                    nc.any.memset(vext[:, :, hp * 65 + 64 : hp * 65 + 65], 1.0)
                for tch in range(NQC):
                    tsl = slice(tch * 512, (tch + 1) * 512)
                    xa = xaug_pool.tile([P, KA, 512], bf16, tag="xa")
                    nc.sync.dma_start(xa, x_aug_r[:, :, tsl])
                    pmu = psA.tile([P, 512], f32, tag="a", name="pmu")
                    psq = psA.tile([P, 512], f32, tag="a", name="psq")
                    for kt in range(KT):
                        xsq = workA.tile([P, 512], bf16, tag="xsq")
                        nc.vector.tensor_tensor(
                            xsq, xa[:, kt, :], xa[:, kt, :], ALU.mult
                        )
                        nc.tensor.matmul(
                            pmu, ones128, xa[:, kt, :],
                            start=(kt == 0), stop=(kt == KT - 1),
                        )
                        nc.tensor.matmul(
                            psq, ones128, xsq,
                            start=(kt == 0), stop=(kt == KT - 1),
                        )
                    m1 = workA.tile([P, 512], f32, tag="m1")
                    nc.vector.tensor_scalar_mul(m1, pmu, 1.0 / D)
                    # augmented row: -mu (bf16), partition 0 of k-tile 8
                    nc.vector.tensor_scalar_mul(xa[0:1, KT, :], m1[0:1, :], -1.0)
                    v1 = workA.tile([P, 512], f32, tag="v1")
                    nc.vector.tensor_scalar_mul(v1, psq, 1.0 / D)
                    m2 = workA.tile([P, 512], f32, tag="m2")
                    nc.vector.tensor_tensor(m2, m1, m1, ALU.mult)
                    nc.vector.tensor_tensor(v1, v1, m2, ALU.subtract)
                    sd = workA.tile([P, 512], f32, tag="sd")
                    nc.scalar.activation(sd, v1, AF.Sqrt, bias=eps_col)
                    r1b = xaug_pool.tile([P, 512], f32, tag="r1b")
                    nc.vector.reciprocal_approx_fast(r1b, sd)

                    for pt in range(2):
                        vtmp = None
                        # v first so its transposes can interleave behind the
                        # q/k matmul groups without stalling the chunk boundary
                        for m in (2, 0, 1):
                            msl = slice(pt * 3 * P + m * P, pt * 3 * P + (m + 1) * P)
                            ps = lgp.tile([P, 512], f32, tag="lg", name="qkvps")
                            for kt in range(KA):
                                nc.tensor.matmul(
                                    ps, wqkv_sb[:, kt, msl], xa[:, kt, :],
                                    start=(kt == 0), stop=(kt == KA - 1),
                                )
                            if m < 2:
                                dst = qkvT[pt][:, m, tsl]
                            else:
                                vtmp = etp.tile(
                                    [P, 512], bf16, tag="et", name=f"vtmp{pt}"
                                )
                                dst = vtmp
                            nc.vector.tensor_tensor(dst, ps, r1b, ALU.mult)
                            if has_c1:
                                nc.vector.tensor_scalar(
                                    dst, dst,
                                    aux_sb[:, 48 + pt * 3 + m : 49 + pt * 3 + m],
                                    None, ALU.add,
                                )
                        with nc.named_scope("vext"):
                            for k4 in range(4):
                                kt = tch * 4 + k4
                                pt_t = psA.tile([P, 512], bf16, tag="a", name="ptt")[
                                    :, 0:P
                                ]
                                nc.tensor.transpose(
                                    pt_t, vtmp[:, k4 * P : (k4 + 1) * P], ident
                                )
                                c0 = pt * 130
                                nc.vector.tensor_copy(
                                    vext[:, kt, c0 : c0 + 64], pt_t[:, 0:64]
                                )
                                nc.vector.tensor_copy(
                                    vext[:, kt, c0 + 65 : c0 + 129], pt_t[:, 64:128]
                                )

            # prefetch residual + MLP-up weights during attention
            nc.sync.dma_start(xc, x_c_d.rearrange("(k p) t -> p k t", p=P))
            FQ = FF // 4
            for q in range(4):
                w1qt = w1_pool.tile([P, KT, FQ], bf16, tag=f"w1_{q}", name=f"w1q{q}")
                nc.sync.dma_start(w1qt, w1_noaug_r[:, :, q * FQ : (q + 1) * FQ])
                w1q.append(w1qt)

            # ---- residual + LN2 pipeline, staged so it can interleave with
            # attention.  SBUF-only elementwise goes to the (idle) GpSimd
            # engine; PSUM reads stay on DVE/Scalar.
            slab_state = {}

            def _slab_s1(s, pool):
                csl = slice(s * SLAB, (s + 1) * SLAB)
                x1p = pool.tile([P, KT, SLAB], bf16, tag="x1p", name=f"x1p{s}")
                # gpsimd SW-DGE queue: keeps this RS-gated load out of the
                # sync queue so a slow collective can't stall proj scatters,
                # and casts the fp8 partial sums back to bf16
                nc.gpsimd.dma_start(
                    out=x1p, in_=x1p_d[s].rearrange("p (k t) -> p k t", k=KT)
                )
                nc.gpsimd.tensor_scalar_mul(x1p, x1p, 1.0 / 64.0)
                xsqs = pool.tile([P, KT, SLAB], bf16, tag="xsqs", name=f"xsqs{s}")
                for kt in range(KT):
                    nc.gpsimd.tensor_tensor(
                        xb[:, kt, csl], xc[:, kt, csl], x1p[:, kt, :], ALU.add
                    )
                    if has_bproj:
                        nc.gpsimd.tensor_scalar(
                            xb[:, kt, csl], xb[:, kt, csl],
                            aux_sb[:, kt : kt + 1], None, ALU.add,
                        )
                    nc.gpsimd.tensor_tensor(
                        xsqs[:, kt, :], xb[:, kt, csl], xb[:, kt, csl], ALU.mult
                    )
                slab_state[s] = xsqs

            def _slab_s2(s):
                csl = slice(s * SLAB, (s + 1) * SLAB)
                xsqs = slab_state[s]
                # pmu/psq share one PSUM bank -> single accumulation group
                stat = psA.tile([P, 512], f32, tag="a", name=f"stat{s}")
                for kt in range(KT):
                    nc.tensor.matmul(
                        stat[:, 0:SLAB], ones128, xb[:, kt, csl],
                        start=(kt == 0), stop=False, skip_group_check=True,
                    )
                    nc.tensor.matmul(
                        stat[:, SLAB : 2 * SLAB], ones128, xsqs[:, kt, :],
                        start=False, stop=(kt == KT - 1), skip_group_check=True,
                    )
                slab_state[s] = stat

            def _slab_s3(s):
                stat = slab_state[s]
                m1 = work.tile([P, SLAB], f32, tag="m1")
                nc.vector.tensor_scalar_mul(m1, stat[:, 0:SLAB], 1.0 / D)
                v1 = work.tile([P, SLAB], f32, tag="v1")
                nc.vector.tensor_scalar_mul(v1, stat[:, SLAB : 2 * SLAB], 1.0 / D)
                m2 = work.tile([P, SLAB], f32, tag="m2")
                nc.gpsimd.tensor_tensor(m2, m1, m1, ALU.mult)
                nc.gpsimd.tensor_tensor(v1, v1, m2, ALU.subtract)
                sd = work.tile([P, SLAB], f32, tag="sd")
                nc.scalar.activation(sd, v1, AF.Sqrt, bias=eps_col)
                r2b = work.tile([P, SLAB], f32, tag="r2b")
                nc.vector.reciprocal_approx_fast(r2b, sd)
                m1b = work.tile([P, SLAB], bf16, tag="m1b")
                nc.gpsimd.tensor_copy(m1b, m1)
                r2s = work.tile([P, SLAB], bf16, tag="r2s")
                nc.gpsimd.tensor_copy(r2s, r2b)
                slab_state[s] = (m1b, r2s)

            def _slab_s4(s):
                csl = slice(s * SLAB, (s + 1) * SLAB)
                m1b, r2s = slab_state.pop(s)
                for kt in range(KT):
                    nc.gpsimd.tensor_tensor(
                        x1aug[:, kt, csl], xb[:, kt, csl], m1b, ALU.subtract
                    )
                    nc.gpsimd.tensor_tensor(
                        x1aug[:, kt, csl], x1aug[:, kt, csl], r2s, ALU.mult
                    )

            # ============ phase B: attention ================================
            # Heads are processed in partition-tile pairs: the two heads of a
            # pair occupy partitions 0:64 / 64:128, so their K=64 logits
            # matmuls land in disjoint PE row groups and run concurrently
            # (row tiling).  AV matmuls lag L steps behind so the softmax exp
            # (split Act/DVE) is off the critical path.
            from collections import deque

            with tc.tile_pool(name="slabA", bufs=1) as slabpA, \
                 nc.named_scope("attn"):
                epi_q = deque()
                epi_bq = deque()
                proj_q = deque()
                pend = deque()
                L = 3

                def _flush_avq():
                    avq, vcol, et, kt = pend.popleft()
                    nc.tensor.matmul(
                        avq, vext[:, kt, vcol], et,
                        start=(kt == 0), stop=(kt == NKT - 1),
                    )

                def _epi_a(st):
                    pt, hp, qc, avq = st
                    rs_sb = attg.tile([1, 512], f32, tag="rsb", name="rs_sb",
                                      bufs=2)
                    nc.scalar.activation(rs_sb, avq[64:65, :], AF.Copy)
                    rc_f = attg.tile([1, 512], f32, tag="rcf", name="rcf",
                                     bufs=2)
                    nc.vector.reciprocal_approx_fast(rc_f, rs_sb)
                    rc_b = attg.tile([1, 512], bf16, tag="rcb", name="rcb",
                                     bufs=2)
                    nc.scalar.activation(rc_b, rc_f, AF.Copy)
                    return (pt, hp, qc, avq, rc_b)

                def _epi_b(st):
                    pt, hp, qc, avq, rc_b = st
                    q0 = qc * 512
                    rbp = lgp.tile([P, 512], f32, tag="lg", name="rbp")[0:64, :]
                    nc.tensor.matmul(
                        rbp, ones128[0:1, 0:64], rc_b, start=True, stop=True
                    )
                    rbs = attg.tile([64, 512], bf16, tag="rbs", name="rbs",
                                    bufs=2)
                    nc.scalar.activation(rbs, rbp, AF.Copy)
                    nc.vector.tensor_tensor(
                        attnT[pt][hp * HD : (hp + 1) * HD, q0 : q0 + 512],
                        avq[0:64, :], rbs, ALU.mult,
                    )

                def _emit_proj():
                    qc, m = proj_q.popleft()
                    tsl = slice(qc * 512, (qc + 1) * 512)
                    ps = psA.tile([P, 512], f32, tag="a", name="projps")
                    for kt2 in range(2):
                        nc.tensor.matmul(
                            ps, wproj_sb[:, kt2, m * P : (m + 1) * P],
                            attnT[kt2][:, tsl], start=(kt2 == 0), stop=(kt2 == 1),
                        )
                    pb = poutp.tile([P, 512], f8, tag="pout", name="pb")
                    nc.scalar.activation(pb, ps, AF.Copy)
                    # scatter: rank r's slab columns -> rows r*128.., col m*128..
                    nc.sync.dma_start(
                        partial_d[qc].rearrange(
                            "(r p) (m t) -> m p r t", r=4, m=KT
                        )[m],
                        pb.rearrange("p (r t) -> p r t", r=4),
                    )
                    if m == KT - 1:
                        with nc.named_scope("reducescatter"):
                            nc.gpsimd.collective_compute(
                                "ReduceScatter",
                                mybir.AluOpType.add,
                                replica_groups=groups,
                                ins=[partial_d[qc][:]],
                                outs=[x1p_d[qc][:]],
                            )

                for qc in range(NQC):
                    for pt in range(2):
                        q0 = qc * 512
                        avqs = [
                            avqp.tile([P, 512], f32, tag="avq",
                                      name=f"avq{qc}{pt}{hp}")[0:65, :]
                            for hp in range(2)
                        ]
                        for kt in range(NKT):
                            ksl = slice(kt * P, (kt + 1) * P)
                            for hp in range(2):
                                hsl = slice(hp * HD, (hp + 1) * HD)
                                lg = lgp.tile([P, 512], f32, tag="lg", name="lg")
                                nc.tensor.matmul(
                                    lg, qkvT[pt][hsl, 1, ksl],
                                    qkvT[pt][hsl, 0, q0 : q0 + 512],
                                    start=True, stop=True,
                                )
                                et = etp.tile([P, 512], bf16, tag="et")
                                if hp == 0 or kt % 8 == 7:
                                    nc.scalar.activation(
                                        et, lg, AF.Exp, scale=1.0 / np.sqrt(HD)
                                    )
                                else:
                                    nc.vector.tensor_scalar(
                                        et.bitcast(i16), lg, EXP_A, EXP_B,
                                        ALU.mult, ALU.add,
                                    )
                                vcol = slice(
                                    pt * 130 + hp * 65, pt * 130 + hp * 65 + 65
                                )
                                pend.append((avqs[hp], vcol, et, kt))
                            while len(pend) > 2 * L:
                                _flush_avq()
                            if kt in (0, 1) and epi_q:
                                epi_bq.append(_epi_a(epi_q.popleft()))
                            if kt in (3, 4) and epi_bq:
                                _epi_b(epi_bq.popleft())
                            if pt == 0 and kt in (5, 7, 9, 11, 13, 15) \
                                    and proj_q:
                                _emit_proj()
                            if pt == 1 and kt in (1, 3) and proj_q:
                                _emit_proj()
                            if qc >= 2:
                                s = qc - 2
                                if pt == 0 and kt == 4:
                                    _slab_s1(s, slabpA)
                                if pt == 1 and kt == 6:
                                    _slab_s2(s)
                                if pt == 1 and kt == 10:
                                    _slab_s3(s)
                                if pt == 1 and kt == 14:
                                    _slab_s4(s)
                        while pend:
                            _flush_avq()
                        for hp in range(2):
                            epi_q.append((pt, hp, qc, avqs[hp]))
                    proj_q.extend((qc, m) for m in range(8))

                while epi_q:
                    epi_bq.append(_epi_a(epi_q.popleft()))
                while epi_bq:
                    _epi_b(epi_bq.popleft())
                with nc.named_scope("proj"):
                    while proj_q:
                        _emit_proj()
                # slab 2: collective long done; runs during early MLP
                with nc.named_scope("x1_ln2"):
                    _slab_s1(2, slabpA)
                    _slab_s2(2)
                    _slab_s3(2)
                    _slab_s4(2)

        # ============ phase C + D: residual/LN2 per slab + 2-pass MLP =====
        w_stack = ExitStack()
        w2_pool = w_stack.enter_context(tc.tile_pool(name="w2pool", bufs=1))
        psB = w_stack.enter_context(tc.tile_pool(name="psB", bufs=1, space="PSUM"))
        H2S = 24  # h2 ring slots (down trails up by 16 f-tiles)
        h2T = w_stack.enter_context(tc.tile_pool(name="h2", bufs=1)).tile(
            [P, H2S, TC], bf16
        )
        slabB = w_stack.enter_context(tc.tile_pool(name="slabB", bufs=1))

        NF = FF // P  # 32 f-tiles
        NQ = NF // 4  # 8 f-tiles per weight quarter
        w2r = w2t_d.rearrange("(k p) d -> p k d", p=P)
        w2q = [None] * 4

        def _w2s(kt, m):
            return w2q[kt // NQ][:, kt % NQ, m * P : (m + 1) * P]

        assert not has_c2, "nonzero ln2_b not supported"
        HTC = TC // 2  # 256 tokens per MLP pass

        def _accs(sfx):
            acc4 = [
                psB.tile([P, 2 * HTC], f32, tag=f"acc{g}", name=f"m2{sfx}{g}")
                for g in range(4)
            ]
            return [
                acc4[m // 2][:, (m % 2) * HTC : (m % 2 + 1) * HTC]
                for m in range(KT)
            ]

        accs = _accs("p")
        with nc.named_scope("mlp"):
            for p_i in range(2):
                t0 = p_i * HTC
                tsl = slice(t0, t0 + HTC)
                if p_i == 1:
                    accs = _accs("q")
                for j in range(NF):
                    if p_i == 0 and j >= NQ and j % NQ == 0:
                        q = j // NQ - 1
                        w2q[q] = w2_pool.tile(
                            [P, NQ, D], bf16, tag=f"w2_{q}", name=f"w2q{q}"
                        )
                        nc.sync.dma_start(w2q[q], w2r[:, q * NQ : (q + 1) * NQ, :])
                    if p_i == 0 and j in (24, 26, 28, 30):
                        # slab 3's residual+LN2, gated on the last collective
                        with nc.named_scope("x1_ln2_s3"):
                            if j == 24:
                                _slab_s1(3, slabB)
                            elif j == 26:
                                _slab_s2(3)
                            elif j == 28:
                                _slab_s3(3)
                            else:
                                _slab_s4(3)
                    w1h = w1q[j // NQ]
                    msl = slice((j % NQ) * P, (j % NQ + 1) * P)
                    ps = psA.tile([P, HTC], f32, tag="a", name="m1ps")
                    for kt in range(KT):
                        nc.tensor.matmul(
                            ps, w1h[:, kt, msl], x1aug[:, kt, tsl],
                            start=(kt == 0), stop=(kt == KT - 1),
                        )
                    bias_arg = aux_sb[:, 8 + j : 9 + j] if has_b1 else 0.0
                    nc.scalar.activation(
                        h2T[:, j % H2S, tsl], ps, AF.Relu, bias=bias_arg
                    )
                    if j >= 2 * NQ:
                        kt2 = j - 2 * NQ
                        for m in range(KT):
                            nc.tensor.matmul(
                                accs[m], _w2s(kt2, m), h2T[:, kt2 % H2S, tsl],
                                start=(kt2 == 0 and m % 2 == 0), stop=False,
                                skip_group_check=True,
                            )
                if p_i == 0:
                    w2q[3] = w2_pool.tile([P, NQ, D], bf16, tag="w2_3", name="w2q3")
                    nc.sync.dma_start(w2q[3], w2r[:, 3 * NQ :, :])
                for kt2 in range(NF - 2 * NQ, NF):
                    for m in range(KT):
                        nc.tensor.matmul(
                            accs[m], _w2s(kt2, m), h2T[:, kt2 % H2S, tsl],
                            start=False,
                            stop=(kt2 == NF - 1 and m % 2 == 1),
                            skip_group_check=True,
                        )
                for m in range(KT):
                    ob = work.tile([P, HTC], f32, tag="ob", bufs=2)
                    nc.vector.tensor_tensor(ob, accs[m], xb[:, m, tsl], ALU.add)
                    if has_b2:
                        nc.vector.tensor_scalar(
                            ob, ob, aux_sb[:, 40 + m : 41 + m], None, ALU.add
                        )
                    nc.sync.dma_start(out_d[m * P : (m + 1) * P, tsl], ob)
        w_stack.close()

    nc.compile()
    return nc


def _slab_cols(c):
    """Column indices into xT [D, T] owned by core c, in kernel order."""
    bc, r = c // 4, c % 4
    cols = []
    for qc in range(NQC):
        base = bc * TB + qc * 512 + r * SLAB
        cols.append(np.arange(base, base + SLAB))
    return np.concatenate(cols)


def _prep_inputs(inputs):
    x = np.asarray(inputs["x"], np.float32)
    w_qkv = np.asarray(inputs["w_qkv"], np.float32)
    w_proj = np.asarray(inputs["w_proj"], np.float32)
    b_proj = np.asarray(inputs["b_proj"], np.float32)
    w1 = np.asarray(inputs["w1"], np.float32)
    b1 = np.asarray(inputs["b1"], np.float32)
    w2 = np.asarray(inputs["w2"], np.float32)
    b2 = np.asarray(inputs["b2"], np.float32)
    ln1_g = np.asarray(inputs["ln1_g"], np.float32)
    ln1_b = np.asarray(inputs["ln1_b"], np.float32)
    ln2_g = np.asarray(inputs["ln2_g"], np.float32)
    ln2_b = np.asarray(inputs["ln2_b"], np.float32)

    has_c1 = bool(np.any(ln1_b != 0))
    has_bproj = bool(np.any(b_proj != 0))
    has_c2 = bool(np.any(ln2_b != 0))
    has_b1 = bool(np.any(b1 != 0))
    has_b2 = bool(np.any(b2 != 0))
    flags = (has_c1, has_bproj, has_c2, has_b1, has_b2)

    xT = np.ascontiguousarray(x.reshape(T, D).T)  # [D, T] f32

    wg = w_qkv * ln1_g[None, :]  # [3D, D]
    Se = wg.sum(axis=1)  # [3D]
    Ce = w_qkv @ ln1_b  # [3D]
    w1g = w1 * ln2_g[None, :]  # [FF, D]
    C2 = w1 @ ln2_b
    if np.any(C2 != 0):
        raise NotImplementedError("nonzero ln2_b not supported")

    w1_aug = np.ascontiguousarray(w1g.T).astype(BF16)
    w2t = np.ascontiguousarray(w2.T).astype(BF16)  # [FF, D]

    in_maps = []
    for c in range(NCORES):
        bc, hg = c // 4, c % 4
        # batch-sliced augmented x
        x_aug = np.zeros((DAUG, TB), BF16)
        x_aug[:D] = xT[:, bc * TB : (bc + 1) * TB].astype(BF16)

        # qkv weights for 4 heads: two partition-tiles of head pairs
        wqkv_aug = np.zeros((DAUG, 6 * P), BF16)
        cstack = np.zeros((P, 6), np.float32)
        for pt in range(2):
            r0 = (4 * hg + 2 * pt) * HD  # 128 contiguous rows (2 heads)
            for m in range(3):
                rows = slice(m * D + r0, m * D + r0 + 2 * HD)
                csl = slice(pt * 3 * P + m * P, pt * 3 * P + (m + 1) * P)
                wqkv_aug[:D, csl] = wg[rows].T.astype(BF16)
                wqkv_aug[D, csl] = Se[rows].astype(BF16)
                cstack[:, pt * 3 + m] = Ce[rows]

        # proj rows for this core's 256 head dims, pre-scaled x64 so the
        # fp8e4m3 proj partials stay in e4m3's good range
        wproj_c = np.ascontiguousarray(
            64.0 * w_proj[:, 4 * hg * HD : (4 * hg + 4) * HD].T
        ).astype(BF16)  # [256, D]

        aux = np.zeros((P, 64), np.float32)
        aux[:, 0:8] = b_proj.reshape(KT, P).T
        aux[:, 8:40] = b1.reshape(FF // P, P).T
        aux[:, 40:48] = b2.reshape(KT, P).T
        aux[:, 48:54] = cstack

        in_maps.append(
            {
                "x_aug": x_aug,
                "x_c": np.ascontiguousarray(xT[:, _slab_cols(c)]).astype(BF16),
                "wqkv_aug": wqkv_aug,
                "wproj_c": wproj_c,
                "w1_aug": w1_aug,
                "w2t": w2t,
                "aux": aux,
            }
        )
    return flags, in_maps


def _run(inputs, trace=False, trace_kwargs=None):
    from concourse.bass_utils import run_bass_kernel_spmd

    flags, in_maps = _prep_inputs(inputs)
    if flags not in _CACHE:
        _CACHE[flags] = _build_program(*flags)
    nc = _CACHE[flags]
    res = run_bass_kernel_spmd(
        nc, in_maps, list(range(NCORES)), trace=trace,
        **(trace_kwargs or {}),
    )
    outT = np.empty((D, T), np.float32)
    for c in range(NCORES):
        outT[:, _slab_cols(c)] = res.results[c]["out_c"]
    out = np.ascontiguousarray(outT.T).reshape(B, S, D)
    return out, res


def kernel(**inputs):
    out, _ = _run(inputs, trace=False)
    return out


# revision 52
# speedup vs baseline: 1.0719x; 1.0719x over previous
"""Trainium2 Bass kernel for a pre-LN transformer block (B=2, S=2048, D=1024,
H=16, d_ff=4096), 8-way (batch, head-group) tensor-parallel:

- core c handles batch c//4 and heads 4*(c%4)..4*(c%4)+3: LN1+qkv run over the
  core's 2048 batch tokens only, attention over 4 heads
- softmax exp is split across engines: even key-tiles use the Activation
  engine's exact Exp, odd key-tiles use a Schraudolph-style int16 exponent
  construction on the DVE (bitcast to bf16)
- attention-proj partials are ReduceScattered per query-chunk (4 collectives),
  each fired as soon as that chunk's proj partials are done, so 3 of 4 overlap
  the remaining attention compute; each core owns four interleaved 128-token
  slabs (slab qc = tokens qc*512 + rank*128 ..+128) so the residual+LN2+MLP
  pipeline starts at attention end, with the MLP split into two 256-token
  passes (the second gated only on the last collective)
- token-sharded MLP with the full d_ff on each core (no second collective)

Activations live feature-major [feature, token].  LayerNorm is folded into the
matmuls via an augmented contraction row (-mu) and column (row-sums of the
g-scaled weights); the 1/sigma factor is applied on PSUM eviction.  Softmax is
computed unnormalized with a ones-column appended to V producing row sums, and
1/sum is applied on the attention-output eviction.
"""

import sys

for _p in ("/opt/trn_rl_repo",):
    if _p not in sys.path:
        sys.path.insert(0, _p)

import numpy as np
import ml_dtypes

B, S, D = 2, 2048, 1024
H, HD = 16, 64
FF = 4 * D
T = B * S  # 4096 tokens
NCORES = 8
TC = T // NCORES  # 512 tokens per core (MLP/out shard)
TB = S  # 2048 tokens per batch (per-core attention range)
P = 128
KT = D // P  # 8 k-tiles over D
KA = 9  # augmented k-tiles
DAUG = D + P  # 1152
EPS = 1e-5
NKT = TB // P  # 16 key tiles per batch
NQC = TB // 512  # 4 q-chunks of 512
SLAB = TC // NQC  # 128 tokens per owned slab
BF16 = ml_dtypes.bfloat16

# Schraudolph exp: bf16 bits ~= round(x*log2(e)*128 + (127*128 - 7.63))
LOG2E = float(np.log2(np.e))
EXP_A = 128.0 * LOG2E / np.sqrt(HD)  # logit scale 1/sqrt(HD) folded in
EXP_B = 127.0 * 128.0 - 7.63
# key tiles using exact Exp on the Activation engine (rest: Schraudolph on DVE)
SC_KT = frozenset({0, 2, 4, 6, 8, 10, 12, 14})

_CACHE = {}


def _build_program(has_c1, has_bproj, has_c2, has_b1, has_b2):
    import concourse.mybir as mybir
    import concourse.tile as tile
    from concourse import bacc
    from concourse.masks import make_identity
    from contextlib import ExitStack

    f32 = mybir.dt.float32
    bf16 = mybir.dt.bfloat16
    f8 = mybir.dt.float8e4
    i16 = mybir.dt.int16
    AF = mybir.ActivationFunctionType
    ALU = mybir.AluOpType

    nc = bacc.Bacc(None, target_bir_lowering=False)

    # ---- I/O ----
    x_aug_d = nc.declare_dram_parameter("x_aug", [DAUG, TB], bf16, isOutput=False)
    x_c_d = nc.declare_dram_parameter("x_c", [D, TC], bf16, isOutput=False)
    wqkv_d = nc.declare_dram_parameter("wqkv_aug", [DAUG, 6 * P], bf16, isOutput=False)
    wproj_d = nc.declare_dram_parameter("wproj_c", [2 * P, D], bf16, isOutput=False)
    w1_d = nc.declare_dram_parameter("w1_aug", [D, FF], bf16, isOutput=False)
    w2t_d = nc.declare_dram_parameter("w2t", [FF, D], bf16, isOutput=False)
    aux_d = nc.declare_dram_parameter("aux", [P, 64], f32, isOutput=False)
    # aux columns: 0:8 -> b_proj as [128,8], 8:40 -> b1 as [128,32],
    # 40:48 -> b2 as [128,8], 48:54 -> C1 (qkv bias-fold) as [128,6]
    out_d = nc.declare_dram_parameter("out_c", [D, TC], f32, isOutput=True)

    groups = [[0, 1, 2, 3], [4, 5, 6, 7]]

    with tile.TileContext(nc) as tc, ExitStack() as ctx:
        const = ctx.enter_context(tc.tile_pool(name="const", bufs=1))
        dram = ctx.enter_context(tc.tile_pool(name="dram", bufs=1, space="DRAM"))

        ident = const.tile([P, P], bf16)
        make_identity(nc, ident)
        ones128 = const.tile([P, P], bf16)
        nc.any.memset(ones128, 1.0)
        eps_col = const.tile([P, 1], f32)
        nc.any.memset(eps_col, EPS)

        wqkv_sb = const.tile([P, KA, 6 * P], bf16)
        nc.sync.dma_start(wqkv_sb, wqkv_d.rearrange("(k p) e -> p k e", p=P))
        wproj_sb = const.tile([P, 2, D], bf16)
        nc.sync.dma_start(wproj_sb, wproj_d.rearrange("(k p) d -> p k d", p=P))
        aux_sb = const.tile([P, 64], f32)
        nc.sync.dma_start(aux_sb, aux_d[:])

        # long-lived activation tensors
        x1grp = ctx.enter_context(tc.tile_pool(name="x1grp", bufs=1))
        x1aug = x1grp.tile([P, KT, TC], bf16)
        work = ctx.enter_context(tc.tile_pool(name="work", bufs=1))

        psA = ctx.enter_context(tc.tile_pool(name="psA", bufs=2, space="PSUM"))

        # residual input, prefetched during attention
        resid = ctx.enter_context(tc.tile_pool(name="resid", bufs=1))
        xc = resid.tile([P, KT, TC], bf16, tag="xc")
        xb = resid.tile([P, KT, TC], bf16, tag="xb")

        # w1 weights, prefetched during attention
        w1_pool = ctx.enter_context(tc.tile_pool(name="w1pool", bufs=1))

        # proj partials per query chunk, wide-row layout for the collective:
        # row r*128 + p, col m*128 + t  <->  feature m*128+p, rank-r slab
        # token t (2KB rows so the ReduceScatter moves efficient lines)
        # fp8 partials (w_proj is pre-scaled x64 on the host so values sit in
        # e4m3's good range); the gpsimd cast-DMA converts back to bf16 and
        # the slab pipeline undoes the x64
        partial_d = [
            dram.tile([4 * P, KT * SLAB], f8, tag=f"pp{qc}", name=f"pp{qc}")
            for qc in range(NQC)
        ]
        x1p_d = [
            dram.tile([P, KT * SLAB], f8, tag=f"xp{qc}", name=f"xp{qc}")
            for qc in range(NQC)
        ]

        x_aug_r = x_aug_d.rearrange("(k p) t -> p k t", p=P)
        w1_noaug_r = w1_d.rearrange("(k p) f -> p k f", p=P)

        w1q = []
        with tc.tile_pool(name="qkvTp", bufs=1) as qkvT_pool, \
             tc.tile_pool(name="attnTp", bufs=1) as attnT_pool, \
             tc.tile_pool(name="attg", bufs=1) as attg, \
             tc.tile_pool(name="etp", bufs=9) as etp, \
             tc.tile_pool(name="poutp", bufs=3) as poutp, \
             tc.tile_pool(name="lgp", bufs=3, space="PSUM") as lgp, \
             tc.tile_pool(name="avqp", bufs=3, space="PSUM") as avqp:
            qkvT = [qkvT_pool.tile([P, 2, TB], bf16, name=f"qkvT{pt}") for pt in (0, 1)]
            attnT = [attnT_pool.tile([P, TB], bf16, name=f"attnT{pt}") for pt in (0, 1)]
            # vext: per key tile: [h0 | 1 | h1 | 1 | h2 | 1 | h3 | 1]
            vext = attg.tile([P, NKT, 4 * 65], bf16)

            # ============ phase A: LN1 stats + qkv + vext, per token chunk ===
            with tc.tile_pool(name="xaug", bufs=2) as xaug_pool, \
                 tc.tile_pool(name="workA", bufs=2) as workA, \
                 nc.named_scope("ln1_qkv"):
                for hp in range(4):
                    nc.any.memset(vext[:, :, hp * 65 + 64 : hp * 65 + 65], 1.0)
                for tch in range(NQC):
                    tsl = slice(tch * 512, (tch + 1) * 512)
                    xa = xaug_pool.tile([P, KA, 512], bf16, tag="xa")
                    nc.sync.dma_start(xa, x_aug_r[:, :, tsl])
                    pmu = psA.tile([P, 512], f32, tag="a", name="pmu")
                    psq = psA.tile([P, 512], f32, tag="a", name="psq")
                    for kt in range(KT):
                        xsq = workA.tile([P, 512], bf16, tag="xsq")
                        nc.vector.tensor_tensor(
                            xsq, xa[:, kt, :], xa[:, kt, :], ALU.mult
                        )
                        nc.tensor.matmul(
                            pmu, ones128, xa[:, kt, :],
                            start=(kt == 0), stop=(kt == KT - 1),
                        )
                        nc.tensor.matmul(
                            psq, ones128, xsq,
                            start=(kt == 0), stop=(kt == KT - 1),
                        )
                    m1 = workA.tile([P, 512], f32, tag="m1")
                    nc.vector.tensor_scalar_mul(m1, pmu, 1.0 / D)
                    # augmented row: -mu (bf16), partition 0 of k-tile 8
                    nc.vector.tensor_scalar_mul(xa[0:1, KT, :], m1[0:1, :], -1.0)
                    v1 = workA.tile([P, 512], f32, tag="v1")
                    nc.vector.tensor_scalar_mul(v1, psq, 1.0 / D)
                    m2 = workA.tile([P, 512], f32, tag="m2")
                    nc.vector.tensor_tensor(m2, m1, m1, ALU.mult)
                    nc.vector.tensor_tensor(v1, v1, m2, ALU.subtract)
                    sd = workA.tile([P, 512], f32, tag="sd")
                    nc.scalar.activation(sd, v1, AF.Sqrt, bias=eps_col)
                    r1b = xaug_pool.tile([P, 512], f32, tag="r1b")
                    nc.vector.reciprocal_approx_fast(r1b, sd)

                    for pt in range(2):
                        vtmp = None
                        # v first so its transposes can interleave behind the
                        # q/k matmul groups without stalling the chunk boundary
                        for m in (2, 0, 1):
                            msl = slice(pt * 3 * P + m * P, pt * 3 * P + (m + 1) * P)
                            ps = lgp.tile([P, 512], f32, tag="lg", name="qkvps")
                            for kt in range(KA):
                                nc.tensor.matmul(
                                    ps, wqkv_sb[:, kt, msl], xa[:, kt, :],
                                    start=(kt == 0), stop=(kt == KA - 1),
                                )
                            if m < 2:
                                dst = qkvT[pt][:, m, tsl]
                            else:
                                vtmp = etp.tile(
                                    [P, 512], bf16, tag="et", name=f"vtmp{pt}"
                                )
                                dst = vtmp
                            nc.vector.tensor_tensor(dst, ps, r1b, ALU.mult)
                            if has_c1:
                                nc.vector.tensor_scalar(
                                    dst, dst,
                                    aux_sb[:, 48 + pt * 3 + m : 49 + pt * 3 + m],
                                    None, ALU.add,
                                )
                        with nc.named_scope("vext"):
                            for k4 in range(4):
                                kt = tch * 4 + k4
                                pt_t = psA.tile([P, 512], bf16, tag="a", name="ptt")[
                                    :, 0:P
                                ]
                                nc.tensor.transpose(
                                    pt_t, vtmp[:, k4 * P : (k4 + 1) * P], ident
                                )
                                c0 = pt * 130
                                nc.vector.tensor_copy(
                                    vext[:, kt, c0 : c0 + 64], pt_t[:, 0:64]
                                )
                                nc.vector.tensor_copy(
                                    vext[:, kt, c0 + 65 : c0 + 129], pt_t[:, 64:128]
                                )

            # prefetch residual + MLP-up weights during attention
            nc.sync.dma_start(xc, x_c_d.rearrange("(k p) t -> p k t", p=P))
            FQ = FF // 4
            for q in range(4):
                w1qt = w1_pool.tile([P, KT, FQ], bf16, tag=f"w1_{q}", name=f"w1q{q}")
                nc.sync.dma_start(w1qt, w1_noaug_r[:, :, q * FQ : (q + 1) * FQ])
                w1q.append(w1qt)

            # ---- residual + LN2 pipeline, staged so it can interleave with
            # attention.  SBUF-only elementwise goes to the (idle) GpSimd
            # engine; PSUM reads stay on DVE/Scalar.
            slab_state = {}
            slab_x1p = {}

            def _slab_load(s, pool):
                x1p = pool.tile([P, KT, SLAB], f8, tag="x1p", name=f"x1p{s}")
                nc.sync.dma_start(
                    x1p, x1p_d[s].rearrange("p (k t) -> p k t", k=KT)
                )
                x1ps = pool.tile([P, KT, SLAB], bf16, tag="x1ps",
                                 name=f"x1ps{s}")
                # one DVE op: fp8 -> bf16 and undo the x64 w_proj pre-scale
                nc.vector.tensor_scalar(x1ps, x1p, 1.0 / 64.0, None, ALU.mult)
                slab_x1p[s] = x1ps

            def _slab_s1(s, pool):
                csl = slice(s * SLAB, (s + 1) * SLAB)
                x1ps = slab_x1p.pop(s)
                xsqs = pool.tile([P, KT, SLAB], bf16, tag="xsqs", name=f"xsqs{s}")
                for kt in range(KT):
                    nc.gpsimd.tensor_tensor(
                        xb[:, kt, csl], xc[:, kt, csl], x1ps[:, kt, :], ALU.add
                    )
                    if has_bproj:
                        nc.gpsimd.tensor_scalar(
                            xb[:, kt, csl], xb[:, kt, csl],
                            aux_sb[:, kt : kt + 1], None, ALU.add,
                        )
                    nc.gpsimd.tensor_tensor(
                        xsqs[:, kt, :], xb[:, kt, csl], xb[:, kt, csl], ALU.mult
                    )
                slab_state[s] = xsqs

            def _slab_s2(s):
                csl = slice(s * SLAB, (s + 1) * SLAB)
                xsqs = slab_state[s]
                # pmu/psq share one PSUM bank -> single accumulation group
                stat = psA.tile([P, 512], f32, tag="a", name=f"stat{s}")
                for kt in range(KT):
                    nc.tensor.matmul(
                        stat[:, 0:SLAB], ones128, xb[:, kt, csl],
                        start=(kt == 0), stop=False, skip_group_check=True,
                    )
                    nc.tensor.matmul(
                        stat[:, SLAB : 2 * SLAB], ones128, xsqs[:, kt, :],
                        start=False, stop=(kt == KT - 1), skip_group_check=True,
                    )
                slab_state[s] = stat

            def _slab_s3(s):
                stat = slab_state[s]
                m1 = work.tile([P, SLAB], f32, tag="m1")
                nc.vector.tensor_scalar_mul(m1, stat[:, 0:SLAB], 1.0 / D)
                v1 = work.tile([P, SLAB], f32, tag="v1")
                nc.vector.tensor_scalar_mul(v1, stat[:, SLAB : 2 * SLAB], 1.0 / D)
                m2 = work.tile([P, SLAB], f32, tag="m2")
                nc.gpsimd.tensor_tensor(m2, m1, m1, ALU.mult)
                nc.gpsimd.tensor_tensor(v1, v1, m2, ALU.subtract)
                sd = work.tile([P, SLAB], f32, tag="sd")
                nc.scalar.activation(sd, v1, AF.Sqrt, bias=eps_col)
                r2b = work.tile([P, SLAB], f32, tag="r2b")
                nc.vector.reciprocal_approx_fast(r2b, sd)
                m1b = work.tile([P, SLAB], bf16, tag="m1b")
                nc.gpsimd.tensor_copy(m1b, m1)
                r2s = work.tile([P, SLAB], bf16, tag="r2s")
                nc.gpsimd.tensor_copy(r2s, r2b)
                slab_state[s] = (m1b, r2s)

            def _slab_s4(s):
                csl = slice(s * SLAB, (s + 1) * SLAB)
                m1b, r2s = slab_state.pop(s)
                for kt in range(KT):
                    nc.gpsimd.tensor_tensor(
                        x1aug[:, kt, csl], xb[:, kt, csl], m1b, ALU.subtract
                    )
                    nc.gpsimd.tensor_tensor(
                        x1aug[:, kt, csl], x1aug[:, kt, csl], r2s, ALU.mult
                    )

            # ============ phase B: attention ================================
            # Heads are processed in partition-tile pairs: the two heads of a
            # pair occupy partitions 0:64 / 64:128, so their K=64 logits
            # matmuls land in disjoint PE row groups and run concurrently
            # (row tiling).  AV matmuls lag L steps behind so the softmax exp
            # (split Act/DVE) is off the critical path.
            from collections import deque

            with tc.tile_pool(name="slabA", bufs=1) as slabpA, \
                 nc.named_scope("attn"):
                epi_q = deque()
                epi_bq = deque()
                proj_q = deque()
                pend = deque()
                L = 3

                def _flush_avq():
                    avq, vcol, et, kt = pend.popleft()
                    nc.tensor.matmul(
                        avq, vext[:, kt, vcol], et,
                        start=(kt == 0), stop=(kt == NKT - 1),
                    )

                def _epi_a(st):
                    pt, hp, qc, avq = st
                    rs_sb = attg.tile([1, 512], f32, tag="rsb", name="rs_sb",
                                      bufs=2)
                    nc.scalar.activation(rs_sb, avq[64:65, :], AF.Copy)
                    rc_f = attg.tile([1, 512], f32, tag="rcf", name="rcf",
                                     bufs=2)
                    nc.vector.reciprocal_approx_fast(rc_f, rs_sb)
                    rc_b = attg.tile([1, 512], bf16, tag="rcb", name="rcb",
                                     bufs=2)
                    nc.scalar.activation(rc_b, rc_f, AF.Copy)
                    return (pt, hp, qc, avq, rc_b)

                def _epi_b(st):
                    pt, hp, qc, avq, rc_b = st
                    q0 = qc * 512
                    rbp = lgp.tile([P, 512], f32, tag="lg", name="rbp")[0:64, :]
                    nc.tensor.matmul(
                        rbp, ones128[0:1, 0:64], rc_b, start=True, stop=True
                    )
                    rbs = attg.tile([64, 512], bf16, tag="rbs", name="rbs",
                                    bufs=2)
                    nc.scalar.activation(rbs, rbp, AF.Copy)
                    nc.vector.tensor_tensor(
                        attnT[pt][hp * HD : (hp + 1) * HD, q0 : q0 + 512],
                        avq[0:64, :], rbs, ALU.mult,
                    )

                def _emit_proj():
                    qc, m = proj_q.popleft()
                    tsl = slice(qc * 512, (qc + 1) * 512)
                    ps = psA.tile([P, 512], f32, tag="a", name="projps")
                    for kt2 in range(2):
                        nc.tensor.matmul(
                            ps, wproj_sb[:, kt2, m * P : (m + 1) * P],
                            attnT[kt2][:, tsl], start=(kt2 == 0), stop=(kt2 == 1),
                        )
                    pb = poutp.tile([P, 512], f8, tag="pout", name="pb")
                    nc.scalar.activation(pb, ps, AF.Copy)
                    # scatter: rank r's slab columns -> rows r*128.., col m*128..
                    nc.sync.dma_start(
                        partial_d[qc].rearrange(
                            "(r p) (m t) -> m p r t", r=4, m=KT
                        )[m],
                        pb.rearrange("p (r t) -> p r t", r=4),
                    )
                    if m == KT - 1:
                        with nc.named_scope("reducescatter"):
                            nc.gpsimd.collective_compute(
                                "ReduceScatter",
                                mybir.AluOpType.add,
                                replica_groups=groups,
                                ins=[partial_d[qc][:]],
                                outs=[x1p_d[qc][:]],
                            )

                for qc in range(NQC):
                    for pt in range(2):
                        q0 = qc * 512
                        avqs = [
                            avqp.tile([P, 512], f32, tag="avq",
                                      name=f"avq{qc}{pt}{hp}")[0:65, :]
                            for hp in range(2)
                        ]
                        for kt in range(NKT):
                            ksl = slice(kt * P, (kt + 1) * P)
                            for hp in range(2):
                                hsl = slice(hp * HD, (hp + 1) * HD)
                                lg = lgp.tile([P, 512], f32, tag="lg", name="lg")
                                nc.tensor.matmul(
                                    lg, qkvT[pt][hsl, 1, ksl],
                                    qkvT[pt][hsl, 0, q0 : q0 + 512],
                                    start=True, stop=True,
                                )
                                et = etp.tile([P, 512], bf16, tag="et")
                                if hp == 0 or kt % 8 == 7:
                                    nc.scalar.activation(
                                        et, lg, AF.Exp, scale=1.0 / np.sqrt(HD)
                                    )
                                else:
                                    nc.vector.tensor_scalar(
                                        et.bitcast(i16), lg, EXP_A, EXP_B,
                                        ALU.mult, ALU.add,
                                    )
                                vcol = slice(
                                    pt * 130 + hp * 65, pt * 130 + hp * 65 + 65
                                )
                                pend.append((avqs[hp], vcol, et, kt))
                            while len(pend) > 2 * L:
                                _flush_avq()
                            if kt in (0, 1) and epi_q:
                                epi_bq.append(_epi_a(epi_q.popleft()))
                            if kt in (3, 4) and epi_bq:
                                _epi_b(epi_bq.popleft())
                            if pt == 0 and kt in (5, 7, 9, 11, 13, 15) \
                                    and proj_q:
                                _emit_proj()
                            if pt == 1 and kt in (1, 3) and proj_q:
                                _emit_proj()
                            if qc >= 2 and pt == 1:
                                s = qc - 2
                                if kt == 5:
                                    _slab_load(s, slabpA)
                                elif kt == 8:
                                    _slab_s1(s, slabpA)
                                elif kt == 12:
                                    _slab_s2(s)
                                elif kt == 15:
                                    _slab_s3(s)
                            if qc == 3 and pt == 0 and kt == 6:
                                _slab_s4(0)
                        while pend:
                            _flush_avq()
                        for hp in range(2):
                            epi_q.append((pt, hp, qc, avqs[hp]))
                    proj_q.extend((qc, m) for m in range(8))

                while epi_q:
                    epi_bq.append(_epi_a(epi_q.popleft()))
                while epi_bq:
                    _epi_b(epi_bq.popleft())
                with nc.named_scope("proj"):
                    while proj_q:
                        _emit_proj()
                # slabs 1 (tail) and 2: collectives long done; the compute
                # runs during early MLP
                with nc.named_scope("x1_ln2"):
                    _slab_s4(1)
                    _slab_load(2, slabpA)
                    _slab_s1(2, slabpA)
                    _slab_s2(2)
                    _slab_s3(2)
                    _slab_s4(2)

        # ============ phase C + D: residual/LN2 per slab + 2-pass MLP =====
        w_stack = ExitStack()
        w2_pool = w_stack.enter_context(tc.tile_pool(name="w2pool", bufs=1))
        psB = w_stack.enter_context(tc.tile_pool(name="psB", bufs=1, space="PSUM"))
        H2S = 24  # h2 ring slots (down trails up by 16 f-tiles)
        h2T = w_stack.enter_context(tc.tile_pool(name="h2", bufs=1)).tile(
            [P, H2S, TC], bf16
        )
        slabB = w_stack.enter_context(tc.tile_pool(name="slabB", bufs=1))

        NF = FF // P  # 32 f-tiles
        NQ = NF // 4  # 8 f-tiles per weight quarter
        w2r = w2t_d.rearrange("(k p) d -> p k d", p=P)
        w2q = [None] * 4

        def _w2s(kt, m):
            return w2q[kt // NQ][:, kt % NQ, m * P : (m + 1) * P]

        assert not has_c2, "nonzero ln2_b not supported"
        HTC = TC // 2  # 256 tokens per MLP pass

        def _accs(sfx):
            acc4 = [
                psB.tile([P, 2 * HTC], f32, tag=f"acc{g}", name=f"m2{sfx}{g}")
                for g in range(4)
            ]
            return [
                acc4[m // 2][:, (m % 2) * HTC : (m % 2 + 1) * HTC]
                for m in range(KT)
            ]

        accs = _accs("p")
        with nc.named_scope("mlp"):
            for p_i in range(2):
                t0 = p_i * HTC
                tsl = slice(t0, t0 + HTC)
                if p_i == 1:
                    accs = _accs("q")
                for j in range(NF):
                    if p_i == 0 and j >= NQ and j % NQ == 0:
                        q = j // NQ - 1
                        w2q[q] = w2_pool.tile(
                            [P, NQ, D], bf16, tag=f"w2_{q}", name=f"w2q{q}"
                        )
                        nc.sync.dma_start(w2q[q], w2r[:, q * NQ : (q + 1) * NQ, :])
                    if p_i == 0 and j in (24, 25, 26, 28, 30):
                        # slab 3's residual+LN2, gated on the last collective
                        with nc.named_scope("x1_ln2_s3"):
                            if j == 24:
                                _slab_load(3, slabB)
                            elif j == 25:
                                _slab_s1(3, slabB)
                            elif j == 26:
                                _slab_s2(3)
                            elif j == 28:
                                _slab_s3(3)
                            else:
                                _slab_s4(3)
                    w1h = w1q[j // NQ]
                    msl = slice((j % NQ) * P, (j % NQ + 1) * P)
                    ps = psA.tile([P, HTC], f32, tag="a", name="m1ps")
                    for kt in range(KT):
                        nc.tensor.matmul(
                            ps, w1h[:, kt, msl], x1aug[:, kt, tsl],
                            start=(kt == 0), stop=(kt == KT - 1),
                        )
                    bias_arg = aux_sb[:, 8 + j : 9 + j] if has_b1 else 0.0
                    nc.scalar.activation(
                        h2T[:, j % H2S, tsl], ps, AF.Relu, bias=bias_arg
                    )
                    if j >= 2 * NQ:
                        kt2 = j - 2 * NQ
                        for m in range(KT):
                            nc.tensor.matmul(
                                accs[m], _w2s(kt2, m), h2T[:, kt2 % H2S, tsl],
                                start=(kt2 == 0 and m % 2 == 0), stop=False,
                                skip_group_check=True,
                            )
                if p_i == 0:
                    w2q[3] = w2_pool.tile([P, NQ, D], bf16, tag="w2_3", name="w2q3")
                    nc.sync.dma_start(w2q[3], w2r[:, 3 * NQ :, :])
                for kt2 in range(NF - 2 * NQ, NF):
                    for m in range(KT):
                        nc.tensor.matmul(
                            accs[m], _w2s(kt2, m), h2T[:, kt2 % H2S, tsl],
                            start=False,
                            stop=(kt2 == NF - 1 and m % 2 == 1),
                            skip_group_check=True,
                        )
                for m in range(KT):
                    ob = work.tile([P, HTC], f32, tag="ob", bufs=2)
                    nc.vector.tensor_tensor(ob, accs[m], xb[:, m, tsl], ALU.add)
                    if has_b2:
                        nc.vector.tensor_scalar(
                            ob, ob, aux_sb[:, 40 + m : 41 + m], None, ALU.add
                        )
                    nc.sync.dma_start(out_d[m * P : (m + 1) * P, tsl], ob)
        w_stack.close()

    nc.compile()
    return nc


def _slab_cols(c):
    """Column indices into xT [D, T] owned by core c, in kernel order."""
    bc, r = c // 4, c % 4
    cols = []
    for qc in range(NQC):
        base = bc * TB + qc * 512 + r * SLAB
        cols.append(np.arange(base, base + SLAB))
    return np.concatenate(cols)


def _prep_inputs(inputs):
    x = np.asarray(inputs["x"], np.float32)
    w_qkv = np.asarray(inputs["w_qkv"], np.float32)
    w_proj = np.asarray(inputs["w_proj"], np.float32)
    b_proj = np.asarray(inputs["b_proj"], np.float32)
    w1 = np.asarray(inputs["w1"], np.float32)
    b1 = np.asarray(inputs["b1"], np.float32)
    w2 = np.asarray(inputs["w2"], np.float32)
    b2 = np.asarray(inputs["b2"], np.float32)
    ln1_g = np.asarray(inputs["ln1_g"], np.float32)
    ln1_b = np.asarray(inputs["ln1_b"], np.float32)
    ln2_g = np.asarray(inputs["ln2_g"], np.float32)
    ln2_b = np.asarray(inputs["ln2_b"], np.float32)

    has_c1 = bool(np.any(ln1_b != 0))
    has_bproj = bool(np.any(b_proj != 0))
    has_c2 = bool(np.any(ln2_b != 0))
    has_b1 = bool(np.any(b1 != 0))
    has_b2 = bool(np.any(b2 != 0))
    flags = (has_c1, has_bproj, has_c2, has_b1, has_b2)

    xT = np.ascontiguousarray(x.reshape(T, D).T)  # [D, T] f32

    wg = w_qkv * ln1_g[None, :]  # [3D, D]
    Se = wg.sum(axis=1)  # [3D]
    Ce = w_qkv @ ln1_b  # [3D]
    w1g = w1 * ln2_g[None, :]  # [FF, D]
    C2 = w1 @ ln2_b
    if np.any(C2 != 0):
        raise NotImplementedError("nonzero ln2_b not supported")

    w1_aug = np.ascontiguousarray(w1g.T).astype(BF16)
    w2t = np.ascontiguousarray(w2.T).astype(BF16)  # [FF, D]

    in_maps = []
    for c in range(NCORES):
        bc, hg = c // 4, c % 4
        # batch-sliced augmented x
        x_aug = np.zeros((DAUG, TB), BF16)
        x_aug[:D] = xT[:, bc * TB : (bc + 1) * TB].astype(BF16)

        # qkv weights for 4 heads: two partition-tiles of head pairs
        wqkv_aug = np.zeros((DAUG, 6 * P), BF16)
        cstack = np.zeros((P, 6), np.float32)
        for pt in range(2):
            r0 = (4 * hg + 2 * pt) * HD  # 128 contiguous rows (2 heads)
            for m in range(3):
                rows = slice(m * D + r0, m * D + r0 + 2 * HD)
                csl = slice(pt * 3 * P + m * P, pt * 3 * P + (m + 1) * P)
                wqkv_aug[:D, csl] = wg[rows].T.astype(BF16)
                wqkv_aug[D, csl] = Se[rows].astype(BF16)
                cstack[:, pt * 3 + m] = Ce[rows]

        # proj rows for this core's 256 head dims, pre-scaled x64 so the
        # fp8e4m3 proj partials stay in e4m3's good range
        wproj_c = np.ascontiguousarray(
            64.0 * w_proj[:, 4 * hg * HD : (4 * hg + 4) * HD].T
        ).astype(BF16)  # [256, D]

        aux = np.zeros((P, 64), np.float32)
        aux[:, 0:8] = b_proj.reshape(KT, P).T
        aux[:, 8:40] = b1.reshape(FF // P, P).T
        aux[:, 40:48] = b2.reshape(KT, P).T
        aux[:, 48:54] = cstack

        in_maps.append(
            {
                "x_aug": x_aug,
                "x_c": np.ascontiguousarray(xT[:, _slab_cols(c)]).astype(BF16),
                "wqkv_aug": wqkv_aug,
                "wproj_c": wproj_c,
                "w1_aug": w1_aug,
                "w2t": w2t,
                "aux": aux,
            }
        )
    return flags, in_maps


def _run(inputs, trace=False, trace_kwargs=None):
    from concourse.bass_utils import run_bass_kernel_spmd

    flags, in_maps = _prep_inputs(inputs)
    if flags not in _CACHE:
        _CACHE[flags] = _build_program(*flags)
    nc = _CACHE[flags]
    res = run_bass_kernel_spmd(
        nc, in_maps, list(range(NCORES)), trace=trace,
        **(trace_kwargs or {}),
    )
    outT = np.empty((D, T), np.float32)
    for c in range(NCORES):
        outT[:, _slab_cols(c)] = res.results[c]["out_c"]
    out = np.ascontiguousarray(outT.T).reshape(B, S, D)
    return out, res


def kernel(**inputs):
    out, _ = _run(inputs, trace=False)
    return out


# revision 55
# speedup vs baseline: 1.0870x; 1.0140x over previous
"""Trainium2 Bass kernel for a pre-LN transformer block (B=2, S=2048, D=1024,
H=16, d_ff=4096), 8-way (batch, head-group) tensor-parallel:

- core c handles batch c//4 and heads 4*(c%4)..4*(c%4)+3: LN1+qkv run over the
  core's 2048 batch tokens only, attention over 4 heads
- softmax exp is split across engines: even key-tiles use the Activation
  engine's exact Exp, odd key-tiles use a Schraudolph-style int16 exponent
  construction on the DVE (bitcast to bf16)
- attention-proj partials are ReduceScattered per query-chunk (4 collectives),
  each fired as soon as that chunk's proj partials are done, so 3 of 4 overlap
  the remaining attention compute; each core owns four interleaved 128-token
  slabs (slab qc = tokens qc*512 + rank*128 ..+128) so the residual+LN2+MLP
  pipeline starts at attention end, with the MLP split into two 256-token
  passes (the second gated only on the last collective)
- token-sharded MLP with the full d_ff on each core (no second collective)

Activations live feature-major [feature, token].  LayerNorm is folded into the
matmuls via an augmented contraction row (-mu) and column (row-sums of the
g-scaled weights); the 1/sigma factor is applied on PSUM eviction.  Softmax is
computed unnormalized with a ones-column appended to V producing row sums, and
1/sum is applied on the attention-output eviction.
"""

import sys

for _p in ("/opt/trn_rl_repo",):
    if _p not in sys.path:
        sys.path.insert(0, _p)

import numpy as np
import ml_dtypes

B, S, D = 2, 2048, 1024
H, HD = 16, 64
FF = 4 * D
T = B * S  # 4096 tokens
NCORES = 8
TC = T // NCORES  # 512 tokens per core (MLP/out shard)
TB = S  # 2048 tokens per batch (per-core attention range)
P = 128
KT = D // P  # 8 k-tiles over D
KA = 9  # augmented k-tiles
DAUG = D + P  # 1152
EPS = 1e-5
NKT = TB // P  # 16 key tiles per batch
NQC = TB // 512  # 4 q-chunks of 512
SLAB = TC // NQC  # 128 tokens per owned slab
BF16 = ml_dtypes.bfloat16

# Schraudolph exp: bf16 bits ~= round(x*log2(e)*128 + (127*128 - 7.63))
LOG2E = float(np.log2(np.e))
EXP_A = 128.0 * LOG2E / np.sqrt(HD)  # logit scale 1/sqrt(HD) folded in
EXP_B = 127.0 * 128.0 - 7.63
# key tiles using exact Exp on the Activation engine (rest: Schraudolph on DVE)
SC_KT = frozenset({0, 2, 4, 6, 8, 10, 12, 14})

_CACHE = {}


def _build_program(has_c1, has_bproj, has_c2, has_b1, has_b2):
    import concourse.mybir as mybir
    import concourse.tile as tile
    from concourse import bacc
    from concourse.masks import make_identity
    from contextlib import ExitStack

    f32 = mybir.dt.float32
    bf16 = mybir.dt.bfloat16
    f8 = mybir.dt.float8e4
    i16 = mybir.dt.int16
    AF = mybir.ActivationFunctionType
    ALU = mybir.AluOpType

    nc = bacc.Bacc(None, target_bir_lowering=False)

    # ---- I/O ----
    x_aug_d = nc.declare_dram_parameter("x_aug", [DAUG, TB], bf16, isOutput=False)
    x_c_d = nc.declare_dram_parameter("x_c", [D, TC], bf16, isOutput=False)
    wqkv_d = nc.declare_dram_parameter("wqkv_aug", [DAUG, 6 * P], bf16, isOutput=False)
    wproj_d = nc.declare_dram_parameter("wproj_c", [2 * P, D], bf16, isOutput=False)
    w1_d = nc.declare_dram_parameter("w1_aug", [D, FF], bf16, isOutput=False)
    w2t_d = nc.declare_dram_parameter("w2t", [FF, D], bf16, isOutput=False)
    aux_d = nc.declare_dram_parameter("aux", [P, 64], f32, isOutput=False)
    # aux columns: 0:8 -> b_proj as [128,8], 8:40 -> b1 as [128,32],
    # 40:48 -> b2 as [128,8], 48:54 -> C1 (qkv bias-fold) as [128,6]
    out_d = nc.declare_dram_parameter("out_c", [D, TC], f32, isOutput=True)

    groups = [[0, 1, 2, 3], [4, 5, 6, 7]]

    with tile.TileContext(nc) as tc, ExitStack() as ctx:
        const = ctx.enter_context(tc.tile_pool(name="const", bufs=1))
        dram = ctx.enter_context(tc.tile_pool(name="dram", bufs=1, space="DRAM"))

        ident = const.tile([P, P], bf16)
        make_identity(nc, ident)
        ones128 = const.tile([P, P], bf16)
        nc.any.memset(ones128, 1.0)
        eps_col = const.tile([P, 1], f32)
        nc.any.memset(eps_col, EPS)

        wqkv_sb = const.tile([P, KA, 6 * P], bf16)
        nc.sync.dma_start(wqkv_sb, wqkv_d.rearrange("(k p) e -> p k e", p=P))
        wproj_sb = const.tile([P, 2, D], bf16)
        nc.sync.dma_start(wproj_sb, wproj_d.rearrange("(k p) d -> p k d", p=P))
        aux_sb = const.tile([P, 64], f32)
        nc.sync.dma_start(aux_sb, aux_d[:])

        # long-lived activation tensors
        x1grp = ctx.enter_context(tc.tile_pool(name="x1grp", bufs=1))
        x1aug = x1grp.tile([P, KT, TC], bf16)
        work = ctx.enter_context(tc.tile_pool(name="work", bufs=1))

        psA = ctx.enter_context(tc.tile_pool(name="psA", bufs=2, space="PSUM"))

        # residual input, prefetched during attention
        resid = ctx.enter_context(tc.tile_pool(name="resid", bufs=1))
        xc = resid.tile([P, KT, TC], bf16, tag="xc")
        xb = resid.tile([P, KT, TC], bf16, tag="xb")

        # w1 weights, prefetched during attention
        w1_pool = ctx.enter_context(tc.tile_pool(name="w1pool", bufs=1))

        # proj partials per query chunk, wide-row layout for the collective:
        # row r*128 + p, col m*128 + t  <->  feature m*128+p, rank-r slab
        # token t (2KB rows so the ReduceScatter moves efficient lines)
        # fp8 partials (w_proj is pre-scaled x64 on the host so values sit in
        # e4m3's good range); the gpsimd cast-DMA converts back to bf16 and
        # the slab pipeline undoes the x64
        partial_d = [
            dram.tile([4 * P, KT * SLAB], f8, tag=f"pp{qc}", name=f"pp{qc}")
            for qc in range(NQC)
        ]
        x1p_d = [
            dram.tile([P, KT * SLAB], f8, tag=f"xp{qc}", name=f"xp{qc}")
            for qc in range(NQC)
        ]

        x_aug_r = x_aug_d.rearrange("(k p) t -> p k t", p=P)
        w1_noaug_r = w1_d.rearrange("(k p) f -> p k f", p=P)

        w1q = []
        with tc.tile_pool(name="qkvTp", bufs=1) as qkvT_pool, \
             tc.tile_pool(name="attnTp", bufs=1) as attnT_pool, \
             tc.tile_pool(name="attg", bufs=1) as attg, \
             tc.tile_pool(name="etp", bufs=9) as etp, \
             tc.tile_pool(name="poutp", bufs=3) as poutp, \
             tc.tile_pool(name="lgp", bufs=4, space="PSUM") as lgp, \
             tc.tile_pool(name="avqp", bufs=2, space="PSUM") as avqp:
            qkvT = [qkvT_pool.tile([P, 2, TB], bf16, name=f"qkvT{pt}") for pt in (0, 1)]
            attnT = [attnT_pool.tile([P, TB], bf16, name=f"attnT{pt}") for pt in (0, 1)]
            # vext: per key tile: [h0 | 1 | h1 | 1 | h2 | 1 | h3 | 1]
            vext = attg.tile([P, NKT, 4 * 65], bf16)

            # ============ phase A: LN1 stats + qkv + vext, per token chunk ===
            with tc.tile_pool(name="xaug", bufs=2) as xaug_pool, \
                 tc.tile_pool(name="workA", bufs=2) as workA, \
                 nc.named_scope("ln1_qkv"):
                for hp in range(4):
                    nc.any.memset(vext[:, :, hp * 65 + 64 : hp * 65 + 65], 1.0)
                for tch in range(NQC):
                    tsl = slice(tch * 512, (tch + 1) * 512)
                    xa = xaug_pool.tile([P, KA, 512], bf16, tag="xa")
                    nc.sync.dma_start(xa, x_aug_r[:, :, tsl])
                    pmu = psA.tile([P, 512], f32, tag="a", name="pmu")
                    psq = psA.tile([P, 512], f32, tag="a", name="psq")
                    for kt in range(KT):
                        xsq = workA.tile([P, 512], bf16, tag="xsq")
                        nc.vector.tensor_tensor(
                            xsq, xa[:, kt, :], xa[:, kt, :], ALU.mult
                        )
                        nc.tensor.matmul(
                            pmu, ones128, xa[:, kt, :],
                            start=(kt == 0), stop=(kt == KT - 1),
                        )
                        nc.tensor.matmul(
                            psq, ones128, xsq,
                            start=(kt == 0), stop=(kt == KT - 1),
                        )
                    m1 = workA.tile([P, 512], f32, tag="m1")
                    nc.vector.tensor_scalar_mul(m1, pmu, 1.0 / D)
                    # augmented row: -mu (bf16), partition 0 of k-tile 8
                    nc.vector.tensor_scalar_mul(xa[0:1, KT, :], m1[0:1, :], -1.0)
                    v1 = workA.tile([P, 512], f32, tag="v1")
                    nc.vector.tensor_scalar_mul(v1, psq, 1.0 / D)
                    m2 = workA.tile([P, 512], f32, tag="m2")
                    nc.vector.tensor_tensor(m2, m1, m1, ALU.mult)
                    nc.vector.tensor_tensor(v1, v1, m2, ALU.subtract)
                    sd = workA.tile([P, 512], f32, tag="sd")
                    nc.scalar.activation(sd, v1, AF.Sqrt, bias=eps_col)
                    r1b = xaug_pool.tile([P, 512], f32, tag="r1b")
                    nc.vector.reciprocal_approx_fast(r1b, sd)

                    for pt in range(2):
                        vtmp = None
                        # v first so its transposes can interleave behind the
                        # q/k matmul groups without stalling the chunk boundary
                        for m in (2, 0, 1):
                            msl = slice(pt * 3 * P + m * P, pt * 3 * P + (m + 1) * P)
                            ps = lgp.tile([P, 512], f32, tag="lg", name="qkvps")
                            for kt in range(KA):
                                nc.tensor.matmul(
                                    ps, wqkv_sb[:, kt, msl], xa[:, kt, :],
                                    start=(kt == 0), stop=(kt == KA - 1),
                                )
                            if m < 2:
                                dst = qkvT[pt][:, m, tsl]
                            else:
                                vtmp = etp.tile(
                                    [P, 512], bf16, tag="et", name=f"vtmp{pt}"
                                )
                                dst = vtmp
                            nc.vector.tensor_tensor(dst, ps, r1b, ALU.mult)
                            if has_c1:
                                nc.vector.tensor_scalar(
                                    dst, dst,
                                    aux_sb[:, 48 + pt * 3 + m : 49 + pt * 3 + m],
                                    None, ALU.add,
                                )
                        with nc.named_scope("vext"):
                            for k4 in range(4):
                                kt = tch * 4 + k4
                                pt_t = psA.tile([P, 512], bf16, tag="a", name="ptt")[
                                    :, 0:P
                                ]
                                nc.tensor.transpose(
                                    pt_t, vtmp[:, k4 * P : (k4 + 1) * P], ident
                                )
                                c0 = pt * 130
                                nc.vector.tensor_copy(
                                    vext[:, kt, c0 : c0 + 64], pt_t[:, 0:64]
                                )
                                nc.vector.tensor_copy(
                                    vext[:, kt, c0 + 65 : c0 + 129], pt_t[:, 64:128]
                                )

            # prefetch residual + MLP-up weights during attention
            nc.sync.dma_start(xc, x_c_d.rearrange("(k p) t -> p k t", p=P))
            FQ = FF // 4
            for q in range(4):
                w1qt = w1_pool.tile([P, KT, FQ], bf16, tag=f"w1_{q}", name=f"w1q{q}")
                nc.sync.dma_start(w1qt, w1_noaug_r[:, :, q * FQ : (q + 1) * FQ])
                w1q.append(w1qt)

            # ---- residual + LN2 pipeline, staged so it can interleave with
            # attention.  SBUF-only elementwise goes to the (idle) GpSimd
            # engine; PSUM reads stay on DVE/Scalar.
            slab_state = {}
            slab_x1p = {}

            def _slab_load(s, pool):
                x1p = pool.tile([P, KT, SLAB], f8, tag="x1p", name=f"x1p{s}")
                nc.sync.dma_start(
                    x1p, x1p_d[s].rearrange("p (k t) -> p k t", k=KT)
                )
                x1ps = pool.tile([P, KT, SLAB], bf16, tag="x1ps",
                                 name=f"x1ps{s}")
                # one DVE op: fp8 -> bf16 and undo the x64 w_proj pre-scale
                nc.vector.tensor_scalar(x1ps, x1p, 1.0 / 64.0, None, ALU.mult)
                slab_x1p[s] = x1ps

            def _slab_s1(s, pool):
                csl = slice(s * SLAB, (s + 1) * SLAB)
                x1ps = slab_x1p.pop(s)
                xsqs = pool.tile([P, KT, SLAB], bf16, tag="xsqs", name=f"xsqs{s}")
                for kt in range(KT):
                    nc.gpsimd.tensor_tensor(
                        xb[:, kt, csl], xc[:, kt, csl], x1ps[:, kt, :], ALU.add
                    )
                    if has_bproj:
                        nc.gpsimd.tensor_scalar(
                            xb[:, kt, csl], xb[:, kt, csl],
                            aux_sb[:, kt : kt + 1], None, ALU.add,
                        )
                    nc.gpsimd.tensor_tensor(
                        xsqs[:, kt, :], xb[:, kt, csl], xb[:, kt, csl], ALU.mult
                    )
                slab_state[s] = xsqs

            def _slab_s2(s):
                csl = slice(s * SLAB, (s + 1) * SLAB)
                xsqs = slab_state[s]
                # pmu/psq share one PSUM bank -> single accumulation group
                stat = psA.tile([P, 512], f32, tag="a", name=f"stat{s}")
                for kt in range(KT):
                    nc.tensor.matmul(
                        stat[:, 0:SLAB], ones128, xb[:, kt, csl],
                        start=(kt == 0), stop=False, skip_group_check=True,
                    )
                    nc.tensor.matmul(
                        stat[:, SLAB : 2 * SLAB], ones128, xsqs[:, kt, :],
                        start=False, stop=(kt == KT - 1), skip_group_check=True,
                    )
                slab_state[s] = stat

            def _slab_s3(s):
                stat = slab_state[s]
                m1 = work.tile([P, SLAB], f32, tag="m1")
                nc.vector.tensor_scalar_mul(m1, stat[:, 0:SLAB], 1.0 / D)
                v1 = work.tile([P, SLAB], f32, tag="v1")
                nc.vector.tensor_scalar_mul(v1, stat[:, SLAB : 2 * SLAB], 1.0 / D)
                m2 = work.tile([P, SLAB], f32, tag="m2")
                nc.gpsimd.tensor_tensor(m2, m1, m1, ALU.mult)
                nc.gpsimd.tensor_tensor(v1, v1, m2, ALU.subtract)
                sd = work.tile([P, SLAB], f32, tag="sd")
                nc.scalar.activation(sd, v1, AF.Sqrt, bias=eps_col)
                r2b = work.tile([P, SLAB], f32, tag="r2b")
                nc.vector.reciprocal_approx_fast(r2b, sd)
                m1b = work.tile([P, SLAB], bf16, tag="m1b")
                nc.gpsimd.tensor_copy(m1b, m1)
                r2s = work.tile([P, SLAB], bf16, tag="r2s")
                nc.gpsimd.tensor_copy(r2s, r2b)
                slab_state[s] = (m1b, r2s)

            def _slab_s4(s):
                csl = slice(s * SLAB, (s + 1) * SLAB)
                m1b, r2s = slab_state.pop(s)
                for kt in range(KT):
                    nc.gpsimd.tensor_tensor(
                        x1aug[:, kt, csl], xb[:, kt, csl], m1b, ALU.subtract
                    )
                    nc.gpsimd.tensor_tensor(
                        x1aug[:, kt, csl], x1aug[:, kt, csl], r2s, ALU.mult
                    )

            # ============ phase B: attention ================================
            # Heads are processed in partition-tile pairs: the two heads of a
            # pair occupy partitions 0:64 / 64:128, so their K=64 logits
            # matmuls land in disjoint PE row groups and run concurrently
            # (row tiling).  AV matmuls lag L steps behind so the softmax exp
            # (split Act/DVE) is off the critical path.
            from collections import deque

            with tc.tile_pool(name="slabA", bufs=1) as slabpA, \
                 nc.named_scope("attn"):
                epi_q = deque()
                epi_bq = deque()
                proj_q = deque()
                pend = deque()
                L = 3

                def _flush_avq():
                    avq, vcol, et, kt = pend.popleft()
                    nc.tensor.matmul(
                        avq, vext[:, kt, vcol], et,
                        start=(kt == 0), stop=(kt == NKT - 1),
                    )

                def _epi_a(st):
                    pt, hp, qc, avq = st
                    rs_sb = attg.tile([1, 512], f32, tag="rsb", name="rs_sb",
                                      bufs=2)
                    nc.scalar.activation(rs_sb, avq[64:65, :], AF.Copy)
                    rc_f = attg.tile([1, 512], f32, tag="rcf", name="rcf",
                                     bufs=2)
                    nc.vector.reciprocal_approx_fast(rc_f, rs_sb)
                    rc_b = attg.tile([1, 512], bf16, tag="rcb", name="rcb",
                                     bufs=2)
                    nc.scalar.activation(rc_b, rc_f, AF.Copy)
                    return (pt, hp, qc, avq, rc_b)

                def _epi_b(st):
                    pt, hp, qc, avq, rc_b = st
                    q0 = qc * 512
                    rbp = lgp.tile([P, 512], f32, tag="lg", name="rbp")[0:64, :]
                    nc.tensor.matmul(
                        rbp, ones128[0:1, 0:64], rc_b, start=True, stop=True
                    )
                    rbs = attg.tile([64, 512], bf16, tag="rbs", name="rbs",
                                    bufs=2)
                    nc.scalar.activation(rbs, rbp, AF.Copy)
                    nc.vector.tensor_tensor(
                        attnT[pt][hp * HD : (hp + 1) * HD, q0 : q0 + 512],
                        avq[0:64, :], rbs, ALU.mult,
                    )

                def _emit_proj():
                    qc, m = proj_q.popleft()
                    tsl = slice(qc * 512, (qc + 1) * 512)
                    ps = psA.tile([P, 512], f32, tag="a", name="projps")
                    for kt2 in range(2):
                        nc.tensor.matmul(
                            ps, wproj_sb[:, kt2, m * P : (m + 1) * P],
                            attnT[kt2][:, tsl], start=(kt2 == 0), stop=(kt2 == 1),
                        )
                    pb = poutp.tile([P, 512], f8, tag="pout", name="pb")
                    nc.scalar.activation(pb, ps, AF.Copy)
                    # scatter: rank r's slab columns -> rows r*128.., col m*128..
                    nc.sync.dma_start(
                        partial_d[qc].rearrange(
                            "(r p) (m t) -> m p r t", r=4, m=KT
                        )[m],
                        pb.rearrange("p (r t) -> p r t", r=4),
                    )
                    if m == KT - 1:
                        with nc.named_scope("reducescatter"):
                            nc.gpsimd.collective_compute(
                                "ReduceScatter",
                                mybir.AluOpType.add,
                                replica_groups=groups,
                                ins=[partial_d[qc][:]],
                                outs=[x1p_d[qc][:]],
                            )

                for qc in range(NQC):
                    for pt in range(2):
                        q0 = qc * 512
                        avqs = [
                            avqp.tile([P, 512], f32, tag="avq",
                                      name=f"avq{qc}{pt}{hp}")[0:65, :]
                            for hp in range(2)
                        ]
                        for kt in range(NKT):
                            ksl = slice(kt * P, (kt + 1) * P)
                            for hp in range(2):
                                hsl = slice(hp * HD, (hp + 1) * HD)
                                lg = lgp.tile([P, 512], f32, tag="lg", name="lg")
                                nc.tensor.matmul(
                                    lg, qkvT[pt][hsl, 1, ksl],
                                    qkvT[pt][hsl, 0, q0 : q0 + 512],
                                    start=True, stop=True,
                                )
                                et = etp.tile([P, 512], bf16, tag="et")
                                if hp == 0 or kt % 8 == 7:
                                    nc.scalar.activation(
                                        et, lg, AF.Exp, scale=1.0 / np.sqrt(HD)
                                    )
                                else:
                                    nc.vector.tensor_scalar(
                                        et.bitcast(i16), lg, EXP_A, EXP_B,
                                        ALU.mult, ALU.add,
                                    )
                                vcol = slice(
                                    pt * 130 + hp * 65, pt * 130 + hp * 65 + 65
                                )
                                pend.append((avqs[hp], vcol, et, kt))
                            while len(pend) > 2 * L:
                                _flush_avq()
                            if kt in (0, 1) and epi_q:
                                epi_bq.append(_epi_a(epi_q.popleft()))
                            if kt in (3, 4) and epi_bq:
                                _epi_b(epi_bq.popleft())
                            if pt == 0 and kt in (5, 7, 9, 11, 13, 15) \
                                    and proj_q:
                                _emit_proj()
                            if pt == 1 and kt in (1, 3) and proj_q:
                                _emit_proj()
                            # slab stages sit ahead of this chunk's CC
                            # trigger (pt1 kt3) in the gpsimd queue -- the
                            # trigger occupies gpsimd for the whole collective
                            if qc >= 2:
                                s = qc - 2
                                if pt == 0:
                                    if kt == 2:
                                        _slab_load(s, slabpA)
                                    elif kt == 6:
                                        _slab_s1(s, slabpA)
                                    elif kt == 12:
                                        _slab_s2(s)
                                elif kt == 0:
                                    _slab_s3(s)
                                elif kt == 8:
                                    _slab_s4(s)
                        while pend:
                            _flush_avq()
                        for hp in range(2):
                            epi_q.append((pt, hp, qc, avqs[hp]))
                    proj_q.extend((qc, m) for m in range(8))

                while epi_q:
                    epi_bq.append(_epi_a(epi_q.popleft()))
                while epi_bq:
                    _epi_b(epi_bq.popleft())
                with nc.named_scope("proj"):
                    while proj_q:
                        _emit_proj()
                # slab 2: collective long done; the compute runs during
                # early MLP
                with nc.named_scope("x1_ln2"):
                    _slab_load(2, slabpA)
                    _slab_s1(2, slabpA)
                    _slab_s2(2)
                    _slab_s3(2)
                    _slab_s4(2)

        # ============ phase C + D: residual/LN2 per slab + 2-pass MLP =====
        w_stack = ExitStack()
        w2_pool = w_stack.enter_context(tc.tile_pool(name="w2pool", bufs=1))
        psB = w_stack.enter_context(tc.tile_pool(name="psB", bufs=1, space="PSUM"))
        H2S = 24  # h2 ring slots (down trails up by 16 f-tiles)
        h2T = w_stack.enter_context(tc.tile_pool(name="h2", bufs=1)).tile(
            [P, H2S, TC], bf16
        )
        slabB = w_stack.enter_context(tc.tile_pool(name="slabB", bufs=1))

        NF = FF // P  # 32 f-tiles
        NQ = NF // 4  # 8 f-tiles per weight quarter
        w2r = w2t_d.rearrange("(k p) d -> p k d", p=P)
        w2q = [None] * 4

        def _w2s(kt, m):
            return w2q[kt // NQ][:, kt % NQ, m * P : (m + 1) * P]

        assert not has_c2, "nonzero ln2_b not supported"
        HTC = TC // 2  # 256 tokens per MLP pass

        def _accs(sfx):
            acc4 = [
                psB.tile([P, 2 * HTC], f32, tag=f"acc{g}", name=f"m2{sfx}{g}")
                for g in range(4)
            ]
            return [
                acc4[m // 2][:, (m % 2) * HTC : (m % 2 + 1) * HTC]
                for m in range(KT)
            ]

        accs = _accs("p")
        with nc.named_scope("mlp"):
            for p_i in range(2):
                t0 = p_i * HTC
                tsl = slice(t0, t0 + HTC)
                if p_i == 1:
                    accs = _accs("q")
                for j in range(NF):
                    if p_i == 0 and j >= NQ and j % NQ == 0:
                        q = j // NQ - 1
                        w2q[q] = w2_pool.tile(
                            [P, NQ, D], bf16, tag=f"w2_{q}", name=f"w2q{q}"
                        )
                        nc.sync.dma_start(w2q[q], w2r[:, q * NQ : (q + 1) * NQ, :])
                    if p_i == 0 and j in (24, 25, 26, 28, 30):
                        # slab 3's residual+LN2, gated on the last collective
                        with nc.named_scope("x1_ln2_s3"):
                            if j == 24:
                                _slab_load(3, slabB)
                            elif j == 25:
                                _slab_s1(3, slabB)
                            elif j == 26:
                                _slab_s2(3)
                            elif j == 28:
                                _slab_s3(3)
                            else:
                                _slab_s4(3)
                    w1h = w1q[j // NQ]
                    msl = slice((j % NQ) * P, (j % NQ + 1) * P)
                    ps = psA.tile([P, HTC], f32, tag="a", name="m1ps")
                    for kt in range(KT):
                        nc.tensor.matmul(
                            ps, w1h[:, kt, msl], x1aug[:, kt, tsl],
                            start=(kt == 0), stop=(kt == KT - 1),
                        )
                    bias_arg = aux_sb[:, 8 + j : 9 + j] if has_b1 else 0.0
                    nc.scalar.activation(
                        h2T[:, j % H2S, tsl], ps, AF.Relu, bias=bias_arg
                    )
                    if j >= 2 * NQ:
                        kt2 = j - 2 * NQ
                        for m in range(KT):
                            nc.tensor.matmul(
                                accs[m], _w2s(kt2, m), h2T[:, kt2 % H2S, tsl],
                                start=(kt2 == 0 and m % 2 == 0), stop=False,
                                skip_group_check=True,
                            )
                if p_i == 0:
                    w2q[3] = w2_pool.tile([P, NQ, D], bf16, tag="w2_3", name="w2q3")
                    nc.sync.dma_start(w2q[3], w2r[:, 3 * NQ :, :])
                for kt2 in range(NF - 2 * NQ, NF):
                    for m in range(KT):
                        nc.tensor.matmul(
                            accs[m], _w2s(kt2, m), h2T[:, kt2 % H2S, tsl],
                            start=False,
                            stop=(kt2 == NF - 1 and m % 2 == 1),
                            skip_group_check=True,
                        )
                for m in range(KT):
                    ob = work.tile([P, HTC], f32, tag="ob", bufs=2)
                    nc.vector.tensor_tensor(ob, accs[m], xb[:, m, tsl], ALU.add)
                    if has_b2:
                        nc.vector.tensor_scalar(
                            ob, ob, aux_sb[:, 40 + m : 41 + m], None, ALU.add
                        )
                    nc.sync.dma_start(out_d[m * P : (m + 1) * P, tsl], ob)
        w_stack.close()

    nc.compile()
    return nc


def _slab_cols(c):
    """Column indices into xT [D, T] owned by core c, in kernel order."""
    bc, r = c // 4, c % 4
    cols = []
    for qc in range(NQC):
        base = bc * TB + qc * 512 + r * SLAB
        cols.append(np.arange(base, base + SLAB))
    return np.concatenate(cols)


def _prep_inputs(inputs):
    x = np.asarray(inputs["x"], np.float32)
    w_qkv = np.asarray(inputs["w_qkv"], np.float32)
    w_proj = np.asarray(inputs["w_proj"], np.float32)
    b_proj = np.asarray(inputs["b_proj"], np.float32)
    w1 = np.asarray(inputs["w1"], np.float32)
    b1 = np.asarray(inputs["b1"], np.float32)
    w2 = np.asarray(inputs["w2"], np.float32)
    b2 = np.asarray(inputs["b2"], np.float32)
    ln1_g = np.asarray(inputs["ln1_g"], np.float32)
    ln1_b = np.asarray(inputs["ln1_b"], np.float32)
    ln2_g = np.asarray(inputs["ln2_g"], np.float32)
    ln2_b = np.asarray(inputs["ln2_b"], np.float32)

    has_c1 = bool(np.any(ln1_b != 0))
    has_bproj = bool(np.any(b_proj != 0))
    has_c2 = bool(np.any(ln2_b != 0))
    has_b1 = bool(np.any(b1 != 0))
    has_b2 = bool(np.any(b2 != 0))
    flags = (has_c1, has_bproj, has_c2, has_b1, has_b2)

    xT = np.ascontiguousarray(x.reshape(T, D).T)  # [D, T] f32

    wg = w_qkv * ln1_g[None, :]  # [3D, D]
    Se = wg.sum(axis=1)  # [3D]
    Ce = w_qkv @ ln1_b  # [3D]
    w1g = w1 * ln2_g[None, :]  # [FF, D]
    C2 = w1 @ ln2_b
    if np.any(C2 != 0):
        raise NotImplementedError("nonzero ln2_b not supported")

    w1_aug = np.ascontiguousarray(w1g.T).astype(BF16)
    w2t = np.ascontiguousarray(w2.T).astype(BF16)  # [FF, D]

    in_maps = []
    for c in range(NCORES):
        bc, hg = c // 4, c % 4
        # batch-sliced augmented x
        x_aug = np.zeros((DAUG, TB), BF16)
        x_aug[:D] = xT[:, bc * TB : (bc + 1) * TB].astype(BF16)

        # qkv weights for 4 heads: two partition-tiles of head pairs
        wqkv_aug = np.zeros((DAUG, 6 * P), BF16)
        cstack = np.zeros((P, 6), np.float32)
        for pt in range(2):
            r0 = (4 * hg + 2 * pt) * HD  # 128 contiguous rows (2 heads)
            for m in range(3):
                rows = slice(m * D + r0, m * D + r0 + 2 * HD)
                csl = slice(pt * 3 * P + m * P, pt * 3 * P + (m + 1) * P)
                wqkv_aug[:D, csl] = wg[rows].T.astype(BF16)
                wqkv_aug[D, csl] = Se[rows].astype(BF16)
                cstack[:, pt * 3 + m] = Ce[rows]

        # proj rows for this core's 256 head dims, pre-scaled x64 so the
        # fp8e4m3 proj partials stay in e4m3's good range
        wproj_c = np.ascontiguousarray(
            64.0 * w_proj[:, 4 * hg * HD : (4 * hg + 4) * HD].T
        ).astype(BF16)  # [256, D]

        aux = np.zeros((P, 64), np.float32)
        aux[:, 0:8] = b_proj.reshape(KT, P).T
        aux[:, 8:40] = b1.reshape(FF // P, P).T
        aux[:, 40:48] = b2.reshape(KT, P).T
        aux[:, 48:54] = cstack

        in_maps.append(
            {
                "x_aug": x_aug,
                "x_c": np.ascontiguousarray(xT[:, _slab_cols(c)]).astype(BF16),
                "wqkv_aug": wqkv_aug,
                "wproj_c": wproj_c,
                "w1_aug": w1_aug,
                "w2t": w2t,
                "aux": aux,
            }
        )
    return flags, in_maps


def _run(inputs, trace=False, trace_kwargs=None):
    from concourse.bass_utils import run_bass_kernel_spmd

    flags, in_maps = _prep_inputs(inputs)
    if flags not in _CACHE:
        _CACHE[flags] = _build_program(*flags)
    nc = _CACHE[flags]
    res = run_bass_kernel_spmd(
        nc, in_maps, list(range(NCORES)), trace=trace,
        **(trace_kwargs or {}),
    )
    outT = np.empty((D, T), np.float32)
    for c in range(NCORES):
        outT[:, _slab_cols(c)] = res.results[c]["out_c"]
    out = np.ascontiguousarray(outT.T).reshape(B, S, D)
    return out, res


def kernel(**inputs):
    out, _ = _run(inputs, trace=False)
    return out
